# revision 23
# baseline (speedup 1.0000x reference)
"""Conv2d 3x3 VALID kernel for Trainium2, batch-sharded across 8 NeuronCores.

Problem: input [32,128,64,64] f32, weights [256,128,3,3] f32 ->
output [32,256,62,62] f32 (stride 1, no padding).

Strategy (per core, 4 images): 1-D Winograd F(2,3) along H + direct taps
along W, matmuls in bf16 (rel-err budget 2e-2, measured ~6e-3).

  For output row pair (2y', 2y'+1), with d_k = in[2y'+k] (rows) and per-kw
  column taps g0..g2 handled by shifted rhs views:
    V0 = d0 - d2, V1 = d1 + d2, V2 = d2 - d1, V3 = d1 - d3   (4 bf16 planes)
    U0 = g0, U1 = (g0+g1+g2)/2, U2 = (g0-g1+g2)/2, U3 = g2   (weights, bf16)
    m_e[y', x] = sum_kw sum_ci U_e,kw[ci,co] * V_e[ci, y', x+kw]  (PE, PSUM)
    out[2y']   = m0 + m1 + m2
    out[2y'+1] = m1 - m2 - m3
  12 matmul passes per 16 output rows instead of the direct method's 18:
  PE row count drops 1.5x.

Engine budget per image (~20us of matmuls):
  ACT:    fp32->bf16 input converts (2) + one 4-plane PSUM->SBUF bf16 copy
          per chunk (the only PSUM reader, so the 4-bank PSUM slot recycles
          in ~1.8us < 2.5us of the next chunk's matmuls; pool bufs=2).
  DVE:    V planes (bf16 2x_1P) + output combines per chunk
          (t_p=c1+c2, t_m=c1-c2, o_even=c0+t_p, o_odd=t_m-c3).
  GPSIMD: nothing. Its SBUF port is shared with the Vector engine and any
          streaming GPSIMD op slows concurrent DVE ops ~8x (measured).

Startup: weight prep for Cout-half 0 is emitted first, then image-0
DMA/convert/V-planes, then half-1 weight prep — so the DVE finishes the
ops that gate the first matmuls before going back to half-1 combos.
"""

import numpy as np

import concourse.bass as bass
import concourse.mybir as mybir
import concourse.tile as tile
from concourse import bacc
from concourse.alu_op_type import AluOpType
from concourse.bass_utils import run_bass_kernel_spmd
from concourse.masks import make_identity

F32 = mybir.dt.float32
BF16 = mybir.dt.bfloat16

B, CIN, H, W = 32, 128, 64, 64
COUT, KH, KW = 256, 3, 3
OH, OW = H - KH + 1, W - KW + 1  # 62, 62
N_CORES = 8
BL = B // N_CORES  # 4 images per core

IMG = H * W  # 4096
W_FREE = CIN * KH * KW  # 1152
N_TAPS = KH * KW  # 9
YT = OH // 2  # 31 y' tiles
VPLANE = YT * W  # 1984 elements per V plane
CHUNKS = [(0, 8), (8, 8), (16, 8), (24, 7)]  # (y'0, n_tiles)


def _weight_prep_taps(nc, tc, psum_pool, w_raw, w_t, u_l, ident, h):
    """PE-transpose half h's taps to [ci, co]; copy out + build u0/u3.

    w_t layout: [ci, tap*256 + h*128 + co]      (fp32, tap = kh*3+kw)
    u_l layout: [ci, (e*3+kw)*256 + h*128 + co] (bf16)
    Transposes are packed 4-per-PSUM-tile (one per bank) so they pipeline
    through the bufs=2 ring instead of serializing on the trailing copies.
    """
    w_v = w_raw[:, h * W_FREE : (h + 1) * W_FREE].rearrange(
        "p (ci t) -> p t ci", t=N_TAPS
    )
    for t0 in range(0, N_TAPS, 4):
        ps = psum_pool.tile([128, 4 * 512], F32, tag="m", name=f"tps_{h}_{t0}")
        for t in range(t0, min(t0 + 4, N_TAPS)):
            nc.tensor.transpose(
                ps[:, (t - t0) * 512 : (t - t0) * 512 + 128], w_v[:, t, :], ident
            )
        for t in range(t0, min(t0 + 4, N_TAPS)):
            nc.vector.tensor_copy(
                w_t[:, t * COUT + h * 128 : t * COUT + h * 128 + 128],
                ps[:, (t - t0) * 512 : (t - t0) * 512 + 128],
            )
    # e0/e3 are plain copies of the kh=0 / kh=2 taps (gate the first MMs)
    for kw in range(KW):
        p0 = w_t[:, (0 + kw) * COUT + h * 128 :][:, :128]
        p2 = w_t[:, (6 + kw) * COUT + h * 128 :][:, :128]
        u0 = u_l[:, (0 * 3 + kw) * COUT + h * 128 :][:, :128]
        u3 = u_l[:, (3 * 3 + kw) * COUT + h * 128 :][:, :128]
        nc.vector.tensor_copy(u0, p0)
        nc.vector.tensor_copy(u3, p2)


def _weight_prep_combos(nc, tc, wtmp_pool, w_t, u_l, h):
    """Build the u1/u2 Winograd combinations for half h (DVE)."""
    for kw in range(KW):
        p0 = w_t[:, (0 + kw) * COUT + h * 128 :][:, :128]
        p1 = w_t[:, (3 + kw) * COUT + h * 128 :][:, :128]
        p2 = w_t[:, (6 + kw) * COUT + h * 128 :][:, :128]
        u1 = u_l[:, (1 * 3 + kw) * COUT + h * 128 :][:, :128]
        u2 = u_l[:, (2 * 3 + kw) * COUT + h * 128 :][:, :128]
        s = wtmp_pool.tile([128, 128], F32, tag="wtmp", name=f"s_{h}_{kw}")
        q = wtmp_pool.tile([128, 128], F32, tag="wtmp", name=f"q_{h}_{kw}")
        nc.vector.tensor_add(s, p0, p2)
        nc.vector.tensor_scalar_mul(q, p1, 0.5)
        nc.vector.scalar_tensor_tensor(
            u1, s, 0.5, q, AluOpType.mult, AluOpType.add
        )
        nc.vector.scalar_tensor_tensor(
            u2, s, 0.5, q, AluOpType.mult, AluOpType.subtract
        )


def _conv_body(nc, tc, out_d, x_d, w_d):
    x_r = x_d.rearrange("b c h w -> b c (h w)")  # [BL, 128, 4096]

    with (
        tc.tile_pool(name="const", bufs=1) as cpool,
        tc.tile_pool(name="psum", bufs=2, space=bass.MemorySpace.PSUM) as psum_pool,
        tc.tile_pool(name="outp", bufs=3) as out_pool,
        tc.tile_pool(name="tmp", bufs=3) as tmp_pool,
    ):
        stage = cpool.tile([128, 2 * IMG], F32)  # rolling fp32 DMA landing
        in_bf = cpool.tile([128, 2 * IMG], BF16)  # rolling bf16 image
        v_all = cpool.tile([128, 2 * 4 * VPLANE], BF16)  # rolling V planes
        w_raw = cpool.tile([128, 2 * W_FREE], F32)
        w_t = cpool.tile([128, N_TAPS * COUT], F32)
        u_l = cpool.tile([128, 12 * COUT], BF16)
        ident = cpool.tile([128, 128], F32)

        make_identity(nc, ident)
        w_r = w_d.rearrange("co ci kh kw -> co (ci kh kw)")  # [256, 1152]
        # Early DMAs issue from the ACT queue: its preamble ends ~1.5us
        # before the sync queue's, so weights + image 0 land sooner.
        nc.scalar.dma_start(
            out=w_raw.rearrange("p (h c) -> p h c", h=2),
            in_=w_r.rearrange("(h p) c -> p h c", h=2),
        )

        def img_dma(b, eng):
            sl = (b % 2) * IMG
            for c0, c1 in ((0, IMG // 2), (IMG // 2, IMG)):
                eng.dma_start(
                    out=stage[:, sl + c0 : sl + c1], in_=x_r[b][:, c0:c1]
                )

        def img_convert(b):
            sl = (b % 2) * IMG
            for c0, c1 in ((0, IMG // 2), (IMG // 2, IMG)):
                nc.scalar.copy(
                    in_bf[:, sl + c0 : sl + c1], stage[:, sl + c0 : sl + c1]
                )

        def v_planes(b, spans):
            dv = in_bf[:, (b % 2) * IMG : (b % 2) * IMG + IMG].rearrange(
                "p (r x) -> p r x", x=W
            )
            for y0, yn in spans:
                r0 = 2 * y0
                rn = 2 * yn

                def rows(k):
                    return dv[:, r0 + k : r0 + k + rn - 1 : 2, :]

                for e, (ra, rb, op) in enumerate(
                    ((0, 2, "sub"), (1, 2, "add"), (2, 1, "sub"), (1, 3, "sub"))
                ):
                    vout = v_all[
                        :,
                        ((b % 2) * 4 + e) * VPLANE
                        + y0 * W : ((b % 2) * 4 + e) * VPLANE
                        + (y0 + yn) * W,
                    ].rearrange("p (y x) -> p y x", x=W)
                    fn = nc.vector.tensor_add if op == "add" else nc.vector.tensor_sub
                    fn(vout, rows(ra), rows(rb))

        def prefetch(b):
            """DMA image b, ACT-convert to bf16, DVE-build V planes (bf16)."""
            img_dma(b, nc.sync)
            img_convert(b)
            v_planes(b, ((0, 31),))

        # Startup interleave: image-0 DMA + convert start on the ACT queue
        # right after the weights DMA; the DVE runs half-0 tap copies, then
        # V0's first half (gates the first matmul), then half-0 combos
        # (needed a few matmuls later), then the rest.
        img_dma(0, nc.scalar)
        img_convert(0)
        _weight_prep_taps(nc, tc, psum_pool, w_raw, w_t, u_l, ident, 0)
        v_planes(0, ((0, 15),))
        _weight_prep_combos(nc, tc, tmp_pool, w_t, u_l, 0)
        v_planes(0, ((15, 16),))
        _weight_prep_taps(nc, tc, psum_pool, w_raw, w_t, u_l, ident, 1)
        _weight_prep_combos(nc, tc, tmp_pool, w_t, u_l, 1)

        for b in range(BL):
            if b + 1 < BL:
                prefetch(b + 1)
            for h in range(2):
                last_bh = b == BL - 1 and h == 1
                chunks = (
                    [(0, 8), (8, 8), (16, 8), (24, 4), (28, 3)]
                    if last_bh
                    else CHUNKS
                )
                for y0, ny in chunks:
                    size = ny * OW
                    m = psum_pool.tile([128, 4 * 512], F32, tag="m", name="m")
                    for e in range(4):
                        vv = v_all[
                            :,
                            ((b % 2) * 4 + e)
                            * VPLANE : ((b % 2) * 4 + e + 1)
                            * VPLANE,
                        ].rearrange("p (y x) -> p y x", x=W)
                        me_v = m[:, e * 512 : e * 512 + size].rearrange(
                            "p (y x) -> p y x", x=OW
                        )
                        for kw in range(KW):
                            lhsT = u_l[:, (e * 3 + kw) * COUT + h * 128 :][
                                :, :128
                            ]
                            nc.tensor.matmul(
                                me_v,
                                lhsT,
                                vv[:, y0 : y0 + ny, kw : kw + OW],
                                start=(kw == 0),
                                stop=(kw == KW - 1),
                            )
                    # Sole PSUM reader: batched 4-plane bf16 copy, so the
                    # 4-bank slot recycles in ~1.8us < the next chunk's MMs.
                    cm = tmp_pool.tile([128, 4 * 496], BF16, tag="cm", name="cm")
                    nc.scalar.copy(
                        cm.rearrange("p (e k) -> p e k", k=496)[:, :, :size],
                        m.rearrange("p (e k) -> p e k", k=512)[:, :, :size],
                    )
                    t12 = tmp_pool.tile(
                        [128, 2 * 496], BF16, tag="t12", name="t12"
                    )
                    c0 = cm[:, 0:size]
                    c1 = cm[:, 496 : 496 + size]
                    c2 = cm[:, 992 : 992 + size]
                    c3 = cm[:, 1488 : 1488 + size]
                    t_p = t12[:, 0:size]
                    t_m = t12[:, 496 : 496 + size]
                    nc.vector.tensor_add(t_p, c1, c2)
                    nc.vector.tensor_sub(t_m, c1, c2)
                    ot = out_pool.tile([128, 16 * OW], F32, name="ot")
                    ot_v = ot[:, : 2 * ny * OW].rearrange("p (y x) -> p y x", x=OW)

                    def v3(ap):
                        return ap.rearrange("p (y x) -> p y x", x=OW)

                    nc.vector.tensor_add(
                        ot_v[:, 0 : 2 * ny : 2, :], v3(c0), v3(t_p)
                    )
                    nc.vector.tensor_sub(
                        ot_v[:, 1 : 2 * ny : 2, :], v3(t_m), v3(c3)
                    )
                    nc.sync.dma_start(
                        out=out_d[
                            b,
                            h * 128 : (h + 1) * 128,
                            2 * y0 : 2 * y0 + 2 * ny,
                            :,
                        ],
                        in_=ot_v,
                    )


def build_module():
    nc = bacc.Bacc(
        "TRN2", target_bir_lowering=False, debug=False, num_devices=N_CORES
    )
    x_d = nc.dram_tensor(
        "input_image", [BL, CIN, H, W], F32, kind="ExternalInput"
    ).ap()
    w_d = nc.dram_tensor("weights", [COUT, CIN, KH, KW], F32, kind="ExternalInput").ap()
    out_d = nc.dram_tensor("out", [BL, COUT, OH, OW], F32, kind="ExternalOutput").ap()
    with tile.TileContext(nc) as tc:
        _conv_body(nc, tc, out_d, x_d, w_d)
    nc.compile()
    return nc


_NC_CACHE = {}


def _get_module():
    if "m" not in _NC_CACHE:
        _NC_CACHE["m"] = build_module()
    return _NC_CACHE["m"]


def kernel(input_image: np.ndarray, weights: np.ndarray) -> np.ndarray:
    input_image = np.ascontiguousarray(input_image, dtype=np.float32)
    weights = np.ascontiguousarray(weights, dtype=np.float32)
    nc = _get_module()
    in_maps = [
        {
            "input_image": input_image[i * BL : (i + 1) * BL],
            "weights": weights,
        }
        for i in range(N_CORES)
    ]
    res = run_bass_kernel_spmd(nc, in_maps, list(range(N_CORES))).results
    return np.concatenate([r["out"] for r in res], axis=0)


# revision 26
# speedup vs baseline: 1.0179x; 1.0179x over previous
"""Conv2d 3x3 VALID kernel for Trainium2, batch-sharded across 8 NeuronCores.

Problem: input [32,128,64,64] f32, weights [256,128,3,3] f32 ->
output [32,256,62,62] f32 (stride 1, no padding).

Strategy (per core, 4 images): 1-D Winograd F(2,3) along H + direct taps
along W, matmuls in bf16 (rel-err budget 2e-2, measured ~6e-3).

  For output row pair (2y', 2y'+1), with d_k = in[2y'+k] (rows) and per-kw
  column taps g0..g2 handled by shifted rhs views:
    V0 = d0 - d2, V1 = d1 + d2, V2 = d2 - d1, V3 = d1 - d3   (4 bf16 planes)
    U0 = g0, U1 = (g0+g1+g2)/2, U2 = (g0-g1+g2)/2, U3 = g2   (weights, bf16)
    m_e[y', x] = sum_kw sum_ci U_e,kw[ci,co] * V_e[ci, y', x+kw]  (PE, PSUM)
    out[2y']   = m0 + m1 + m2
    out[2y'+1] = m1 - m2 - m3
  12 matmul passes per 16 output rows instead of the direct method's 18:
  PE row count drops 1.5x.

Engine budget per image (~20us of matmuls):
  ACT:    fp32->bf16 input converts (2) + one 4-plane PSUM->SBUF bf16 copy
          per chunk (the only PSUM reader, so the 4-bank PSUM slot recycles
          in ~1.8us < 2.5us of the next chunk's matmuls; pool bufs=2).
  DVE:    V planes (bf16 2x_1P) + output combines per chunk
          (t_p=c1+c2, t_m=c1-c2, o_even=c0+t_p, o_odd=t_m-c3).
  GPSIMD: nothing. Its SBUF port is shared with the Vector engine and any
          streaming GPSIMD op slows concurrent DVE ops ~8x (measured).

Startup: weight prep for Cout-half 0 is emitted first, then image-0
DMA/convert/V-planes, then half-1 weight prep — so the DVE finishes the
ops that gate the first matmuls before going back to half-1 combos.
"""

import numpy as np

import concourse.bass as bass
import concourse.mybir as mybir
import concourse.tile as tile
from concourse import bacc
from concourse.alu_op_type import AluOpType
from concourse.bass_utils import run_bass_kernel_spmd
from concourse.masks import make_identity

F32 = mybir.dt.float32
BF16 = mybir.dt.bfloat16

B, CIN, H, W = 32, 128, 64, 64
COUT, KH, KW = 256, 3, 3
OH, OW = H - KH + 1, W - KW + 1  # 62, 62
N_CORES = 8
BL = B // N_CORES  # 4 images per core

IMG = H * W  # 4096
W_FREE = CIN * KH * KW  # 1152
N_TAPS = KH * KW  # 9
YT = OH // 2  # 31 y' tiles
VPLANE = YT * W  # 1984 elements per V plane
CHUNKS = [(0, 8), (8, 8), (16, 8), (24, 7)]  # (y'0, n_tiles)


def _weight_prep_taps(nc, tc, psum_pool, w_raw, w_t, u_l, ident, h):
    """PE-transpose half h's taps to [ci, co]; copy out + build u0/u3.

    w_t layout: [ci, tap*256 + h*128 + co]      (fp32, tap = kh*3+kw)
    u_l layout: [ci, (e*3+kw)*256 + h*128 + co] (bf16)
    Transposes are packed 4-per-PSUM-tile (one per bank) so they pipeline
    through the bufs=2 ring instead of serializing on the trailing copies.
    """
    w_v = w_raw[:, h * W_FREE : (h + 1) * W_FREE].rearrange(
        "p (ci t) -> p t ci", t=N_TAPS
    )
    for t0 in range(0, N_TAPS, 4):
        ps = psum_pool.tile([128, 4 * 512], F32, tag="m", name=f"tps_{h}_{t0}")
        for t in range(t0, min(t0 + 4, N_TAPS)):
            nc.tensor.transpose(
                ps[:, (t - t0) * 512 : (t - t0) * 512 + 128], w_v[:, t, :], ident
            )
        for t in range(t0, min(t0 + 4, N_TAPS)):
            # Tap copies run on ACT, keeping the DVE startup chain short.
            nc.scalar.copy(
                w_t[:, t * COUT + h * 128 : t * COUT + h * 128 + 128],
                ps[:, (t - t0) * 512 : (t - t0) * 512 + 128],
            )


def _weight_prep_u03(nc, tc, w_t, u_l, h):
    """u0/u3 are plain bf16 copies of the kh=0/kh=2 taps (gate first MMs)."""
    for kw in range(KW):
        p0 = w_t[:, (0 + kw) * COUT + h * 128 :][:, :128]
        p2 = w_t[:, (6 + kw) * COUT + h * 128 :][:, :128]
        u0 = u_l[:, (0 * 3 + kw) * COUT + h * 128 :][:, :128]
        u3 = u_l[:, (3 * 3 + kw) * COUT + h * 128 :][:, :128]
        nc.vector.tensor_copy(u0, p0)
        nc.vector.tensor_copy(u3, p2)


def _weight_prep_combos(nc, tc, wtmp_pool, w_t, u_l, h):
    """Build the u1/u2 Winograd combinations for half h (DVE)."""
    for kw in range(KW):
        p0 = w_t[:, (0 + kw) * COUT + h * 128 :][:, :128]
        p1 = w_t[:, (3 + kw) * COUT + h * 128 :][:, :128]
        p2 = w_t[:, (6 + kw) * COUT + h * 128 :][:, :128]
        u1 = u_l[:, (1 * 3 + kw) * COUT + h * 128 :][:, :128]
        u2 = u_l[:, (2 * 3 + kw) * COUT + h * 128 :][:, :128]
        s = wtmp_pool.tile([128, 128], F32, tag="wtmp", name=f"s_{h}_{kw}")
        q = wtmp_pool.tile([128, 128], F32, tag="wtmp", name=f"q_{h}_{kw}")
        nc.vector.tensor_add(s, p0, p2)
        nc.vector.tensor_scalar_mul(q, p1, 0.5)
        nc.vector.scalar_tensor_tensor(
            u1, s, 0.5, q, AluOpType.mult, AluOpType.add
        )
        nc.vector.scalar_tensor_tensor(
            u2, s, 0.5, q, AluOpType.mult, AluOpType.subtract
        )


def _conv_body(nc, tc, out_d, x_d, w_d):
    x_r = x_d.rearrange("b c h w -> b c (h w)")  # [BL, 128, 4096]

    with (
        tc.tile_pool(name="const", bufs=1) as cpool,
        tc.tile_pool(name="psum", bufs=2, space=bass.MemorySpace.PSUM) as psum_pool,
        tc.tile_pool(name="outp", bufs=3) as out_pool,
        tc.tile_pool(name="tmp", bufs=3) as tmp_pool,
    ):
        stage = cpool.tile([128, 2 * IMG], F32)  # rolling fp32 DMA landing
        in_bf = cpool.tile([128, 2 * IMG], BF16)  # rolling bf16 image
        v_all = cpool.tile([128, 2 * 4 * VPLANE], BF16)  # rolling V planes
        w_raw = cpool.tile([128, 2 * W_FREE], F32)
        w_t = cpool.tile([128, N_TAPS * COUT], F32)
        u_l = cpool.tile([128, 12 * COUT], BF16)
        ident = cpool.tile([128, 128], F32)

        make_identity(nc, ident)
        w_r = w_d.rearrange("co ci kh kw -> co (ci kh kw)")  # [256, 1152]
        nc.sync.dma_start(
            out=w_raw.rearrange("p (h c) -> p h c", h=2),
            in_=w_r.rearrange("(h p) c -> p h c", h=2),
        )

        def img_dma(b, eng):
            sl = (b % 2) * IMG
            for c0, c1 in ((0, IMG // 2), (IMG // 2, IMG)):
                eng.dma_start(
                    out=stage[:, sl + c0 : sl + c1], in_=x_r[b][:, c0:c1]
                )

        def img_convert(b):
            sl = (b % 2) * IMG
            for c0, c1 in ((0, IMG // 2), (IMG // 2, IMG)):
                nc.scalar.copy(
                    in_bf[:, sl + c0 : sl + c1], stage[:, sl + c0 : sl + c1]
                )

        def v_planes(b, spans):
            dv = in_bf[:, (b % 2) * IMG : (b % 2) * IMG + IMG].rearrange(
                "p (r x) -> p r x", x=W
            )
            for y0, yn in spans:
                r0 = 2 * y0
                rn = 2 * yn

                def rows(k):
                    return dv[:, r0 + k : r0 + k + rn - 1 : 2, :]

                for e, (ra, rb, op) in enumerate(
                    ((0, 2, "sub"), (1, 2, "add"), (2, 1, "sub"), (1, 3, "sub"))
                ):
                    vout = v_all[
                        :,
                        ((b % 2) * 4 + e) * VPLANE
                        + y0 * W : ((b % 2) * 4 + e) * VPLANE
                        + (y0 + yn) * W,
                    ].rearrange("p (y x) -> p y x", x=W)
                    fn = nc.vector.tensor_add if op == "add" else nc.vector.tensor_sub
                    fn(vout, rows(ra), rows(rb))

        def prefetch(b):
            """DMA image b, ACT-convert to bf16, DVE-build V planes (bf16)."""
            img_dma(b, nc.sync)
            img_convert(b)
            v_planes(b, ((0, 31),))

        # Startup interleave: image-0 DMA/convert first (ACT), tap copies on
        # ACT behind them; the DVE runs V0's first half (gates the first
        # matmul), then u0/u3, then half-0 combos (needed a few matmuls
        # later), then the rest.
        img_dma(0, nc.sync)
        img_convert(0)
        _weight_prep_taps(nc, tc, psum_pool, w_raw, w_t, u_l, ident, 0)
        v_planes(0, ((0, 15),))
        _weight_prep_u03(nc, tc, w_t, u_l, 0)
        _weight_prep_combos(nc, tc, tmp_pool, w_t, u_l, 0)
        v_planes(0, ((15, 16),))
        _weight_prep_taps(nc, tc, psum_pool, w_raw, w_t, u_l, ident, 1)
        _weight_prep_u03(nc, tc, w_t, u_l, 1)
        _weight_prep_combos(nc, tc, tmp_pool, w_t, u_l, 1)

        for b in range(BL):
            if b + 1 < BL:
                prefetch(b + 1)
            for h in range(2):
                last_bh = b == BL - 1 and h == 1
                chunks = (
                    [(0, 8), (8, 8), (16, 8), (24, 4), (28, 3)]
                    if last_bh
                    else CHUNKS
                )
                for y0, ny in chunks:
                    size = ny * OW
                    m = psum_pool.tile([128, 4 * 512], F32, tag="m", name="m")
                    for e in range(4):
                        vv = v_all[
                            :,
                            ((b % 2) * 4 + e)
                            * VPLANE : ((b % 2) * 4 + e + 1)
                            * VPLANE,
                        ].rearrange("p (y x) -> p y x", x=W)
                        me_v = m[:, e * 512 : e * 512 + size].rearrange(
                            "p (y x) -> p y x", x=OW
                        )
                        for kw in range(KW):
                            lhsT = u_l[:, (e * 3 + kw) * COUT + h * 128 :][
                                :, :128
                            ]
                            nc.tensor.matmul(
                                me_v,
                                lhsT,
                                vv[:, y0 : y0 + ny, kw : kw + OW],
                                start=(kw == 0),
                                stop=(kw == KW - 1),
                            )
                    # Sole PSUM reader: batched 4-plane bf16 copy, so the
                    # 4-bank slot recycles in ~1.8us < the next chunk's MMs.
                    cm = tmp_pool.tile([128, 4 * 496], BF16, tag="cm", name="cm")
                    nc.scalar.copy(
                        cm.rearrange("p (e k) -> p e k", k=496)[:, :, :size],
                        m.rearrange("p (e k) -> p e k", k=512)[:, :, :size],
                    )
                    t12 = tmp_pool.tile(
                        [128, 2 * 496], BF16, tag="t12", name="t12"
                    )
                    c0 = cm[:, 0:size]
                    c1 = cm[:, 496 : 496 + size]
                    c2 = cm[:, 992 : 992 + size]
                    c3 = cm[:, 1488 : 1488 + size]
                    t_p = t12[:, 0:size]
                    t_m = t12[:, 496 : 496 + size]
                    nc.vector.tensor_add(t_p, c1, c2)
                    nc.vector.tensor_sub(t_m, c1, c2)
                    ot = out_pool.tile([128, 16 * OW], F32, name="ot")
                    ot_v = ot[:, : 2 * ny * OW].rearrange("p (y x) -> p y x", x=OW)

                    def v3(ap):
                        return ap.rearrange("p (y x) -> p y x", x=OW)

                    nc.vector.tensor_add(
                        ot_v[:, 0 : 2 * ny : 2, :], v3(c0), v3(t_p)
                    )
                    nc.vector.tensor_sub(
                        ot_v[:, 1 : 2 * ny : 2, :], v3(t_m), v3(c3)
                    )
                    nc.sync.dma_start(
                        out=out_d[
                            b,
                            h * 128 : (h + 1) * 128,
                            2 * y0 : 2 * y0 + 2 * ny,
                            :,
                        ],
                        in_=ot_v,
                    )


def build_module():
    nc = bacc.Bacc(
        "TRN2", target_bir_lowering=False, debug=False, num_devices=N_CORES
    )
    x_d = nc.dram_tensor(
        "input_image", [BL, CIN, H, W], F32, kind="ExternalInput"
    ).ap()
    w_d = nc.dram_tensor("weights", [COUT, CIN, KH, KW], F32, kind="ExternalInput").ap()
    out_d = nc.dram_tensor("out", [BL, COUT, OH, OW], F32, kind="ExternalOutput").ap()
    with tile.TileContext(nc) as tc:
        _conv_body(nc, tc, out_d, x_d, w_d)
    nc.compile()
    return nc


_NC_CACHE = {}


def _get_module():
    if "m" not in _NC_CACHE:
        _NC_CACHE["m"] = build_module()
    return _NC_CACHE["m"]


def kernel(input_image: np.ndarray, weights: np.ndarray) -> np.ndarray:
    input_image = np.ascontiguousarray(input_image, dtype=np.float32)
    weights = np.ascontiguousarray(weights, dtype=np.float32)
    nc = _get_module()
    in_maps = [
        {
            "input_image": input_image[i * BL : (i + 1) * BL],
            "weights": weights,
        }
        for i in range(N_CORES)
    ]
    res = run_bass_kernel_spmd(nc, in_maps, list(range(N_CORES))).results
    return np.concatenate([r["out"] for r in res], axis=0)
